# revision 24
# baseline (speedup 1.0000x reference)
"""Self-contained Trainium2 Bass kernel for nn_LunarCausalAttention (v2).

Sharding: 8 cores = 2 batches x 4 head-blocks (4 heads each). Params sliced
per core host-side; per-core partial outputs (over head-blocks) summed on
host during the gather (plus bo). Output is bf16 on device, f32 on host.

v2 restructure vs v1: parallel-prefix chunk scan (phase C computes all
per-chunk state deltas independently; a small serial DVE prefix produces
per-chunk states; phase E consumes them chunk-independently), block-diagonal
padded S2 so inter-chunk terms chain into the intra PSUM accumulation group,
out1 computed transposed (cheap 32-col LDWEIGHTS) + one PE transpose,
single-DMA weight loads, bf16 output DMA.

PSUM tags (8 banks): tA lin/pattn/M1, tD lin/weff/attn, tE awp/M2-even,
tV M2-odd, tT phaseC transposes, tS beT/dS, tU pq/o1, tF proj.
"""

import math

import ml_dtypes
import numpy as np

import concourse.bacc as bacc
import concourse.bass as bass
import concourse.mybir as mybir
import concourse.tile as tile

EMBED = 1024
D = 64
PLEN = 32
NTOK = 2048
BSZ = 2
SCALING = D ** -0.5
BETA = math.log(2.0)

NH = 4           # heads per core
C = 128          # chunk (token tile)
NCH = NTOK // C  # 16 chunks
F32 = mybir.dt.float32
BF16 = mybir.dt.bfloat16
AX = mybir.AxisListType
AF = mybir.ActivationFunctionType

# smalls_f32 column layout
SF_BQC = 0          # [128, 4]
SF_BPQ = 4          # [64, 4] (head-major bpq at partitions 0-63)
SF_RLEN = 8         # [128, 16]
SF_MASK = 24        # [128, 128]
SF_COLS = 152
# smalls_bf16 column layout
SB_ID128 = 0        # [128, 128] bf16 identity
SB_BPC = 128        # [64, 4] bpc (heads cols)
SB_COLS = 132


def _bcast(ap_obj, dim_count, at=1):
    """Insert a stride-0 dim of size dim_count into an AP at free position."""
    pat = [list(p) for p in ap_obj.ap]
    pat.insert(at, [0, dim_count])
    return bass.AP(tensor=ap_obj.tensor, offset=ap_obj.offset, ap=pat)


def build_nc(stage=6):
    nc = bacc.Bacc("TRN2", target_bir_lowering=False, debug=False,
                   num_devices=8)

    xT_d = nc.dram_tensor("xT", [EMBED, NTOK], BF16, kind="ExternalInput")
    pxT_d = nc.dram_tensor("pxT", [EMBED, PLEN], BF16, kind="ExternalInput")
    wqc_d = nc.dram_tensor("wqcT", [EMBED, 4 * C], BF16, kind="ExternalInput")
    wpq_d = nc.dram_tensor("wpqT", [EMBED, 2 * C], BF16, kind="ExternalInput")
    wpc_d = nc.dram_tensor("wpcR", [D, NH, 8, 128], BF16, kind="ExternalInput")
    wo_d = nc.dram_tensor("woT", [NH * D, EMBED], BF16, kind="ExternalInput")
    sf_d = nc.dram_tensor("smf", [128, SF_COLS], F32, kind="ExternalInput")
    sb_d = nc.dram_tensor("smb", [128, SB_COLS], BF16, kind="ExternalInput")
    out_d = nc.dram_tensor("out", [NTOK, EMBED], BF16, kind="ExternalOutput")

    with tile.TileContext(nc) as tc:
        with (
            tc.tile_pool(name="big", bufs=1) as big,
            tc.tile_pool(name="work", bufs=3) as work,
            tc.tile_pool(name="outp", bufs=2) as outp,
            tc.tile_pool(name="psp", bufs=1, space="PSUM") as psp,
        ):
            # ---- persistent loads (each a single DMA) ----
            # order: xT q0 + pq-path weights first (critical path), rest after
            wqc = big.tile([128, 8, 4 * C], BF16)
            nc.sync.dma_start(out=wqc,
                              in_=wqc_d.rearrange("(k p) m -> p k m", p=128))
            xT = big.tile([128, 8, NTOK], BF16)
            xT_r = xT_d.rearrange("(k p) n -> p k n", p=128)
            NQ = NTOK // 4
            nc.sync.dma_start(out=xT[:, :, 0:NQ], in_=xT_r[:, :, 0:NQ])
            wpq = big.tile([128, 8, 2 * C], BF16)
            nc.sync.dma_start(out=wpq,
                              in_=wpq_d.rearrange("(k p) m -> p k m", p=128))
            pxT = big.tile([128, 8, PLEN], BF16)
            nc.sync.dma_start(out=pxT,
                              in_=pxT_d.rearrange("(k p) n -> p k n", p=128))
            smf = big.tile([128, SF_COLS], F32)
            nc.sync.dma_start(out=smf, in_=sf_d.ap())
            smb = big.tile([128, SB_COLS], BF16)
            nc.sync.dma_start(out=smb, in_=sb_d.ap())
            wpc = big.tile([D, NH, 8, 128], BF16)
            nc.sync.dma_start(out=wpc, in_=wpc_d.ap())
            wo = big.tile([128, 2, EMBED], BF16)
            nc.sync.dma_start(out=wo,
                              in_=wo_d.rearrange("(k p) o -> p k o", p=128))
            for qi in range(1, 4):
                nc.sync.dma_start(out=xT[:, :, qi * NQ:(qi + 1) * NQ],
                                  in_=xT_r[:, :, qi * NQ:(qi + 1) * NQ])

            bqc = smf[:, SF_BQC:SF_BQC + 4]
            bpq = smf[0:D, SF_BPQ:SF_BPQ + NH]
            rlen = smf[:, SF_RLEN:SF_RLEN + NCH]
            mask = smf[:, SF_MASK:SF_MASK + C]
            id128 = smb[:, SB_ID128:SB_ID128 + 128]
            bpc0 = smb[0:D, SB_BPC:SB_BPC + NH]

            # ---- persistent compute tensors ----
            lin = big.tile([128, 4, NTOK], BF16)      # q(0,1) kv(2,3) chan-major
            lin0 = big.tile([D, 4, NTOK], BF16)       # odd halves at base 0
            z_cm = big.tile([128, NTOK], BF16)        # [(h,p), tok]
            weff = big.tile([128, 8, NH * PLEN], BF16)
            pq0 = big.tile([D, NH, PLEN], BF16)
            beT = big.tile([128, 1], F32)             # beta * bias_eff per part
            zk = big.tile([128, NCH, 3, C], BF16)     # [tok, c, {z,kv0,kv1}]
            dS_sb = big.tile([128, NCH, 192], F32)    # [0:64]=dS2, [64:192]=dS1
            Scum = big.tile([128, 2, 192], F32)
            S1b = big.tile([D, NCH, NH * PLEN], BF16)  # prefix thru c
            S2b = big.tile([128, NCH, NH, D], BF16)    # block-diag padded

            nc.vector.memset(S2b, 0.0)
            nc.vector.memset(dS_sb[D:128, :, 64:192], 0.0)

            def q_at0(h, tok):
                g, half = h // 2, h % 2
                return (lin0[:, g, tok] if half else lin[0:D, g, tok])

            def kv_at0(h, tok):
                g, half = h // 2, h % 2
                return (lin0[:, 2 + g, tok] if half else lin[0:D, 2 + g, tok])

            # ---- pq linear, per head directly at partitions 0-63 ----
            pq_ps = psp.tile([D, NH, PLEN], F32, tag="tU", name="pq_ps",
                             bufs=2)
            for h in range(NH):
                for k in range(8):
                    nc.tensor.matmul(pq_ps[:, h, :],
                                     lhsT=wpq[:, k, h * D:(h + 1) * D],
                                     rhs=pxT[:, k, :],
                                     start=(k == 0), stop=(k == 7))
            for h in range(NH):
                nc.scalar.activation(out=pq0[:, h, :], in_=pq_ps[:, h, :],
                                     func=AF.Identity, bias=bpq[:, h:h + 1],
                                     scale=1.0)

            # beT[(h,p)] = beta * (bpc_h . pq_h[:, p])  (col-packed matmuls)
            beT_ps = psp.tile([128, 512], F32, tag="tU", name="beT_ps",
                              bufs=2)
            for h in range(NH):
                nc.tensor.matmul(beT_ps[32 * h:32 * h + 32, 0:1],
                                 lhsT=pq0[:, h, :], rhs=bpc0[:, h:h + 1],
                                 start=True, stop=True,
                                 tile_position=(0, 32 * h))
            nc.vector.tensor_scalar_mul(beT, beT_ps[:, 0:1], BETA)

            # ---- W_eff[e, (h,p)] = sum_d Wpc[(h,d), e] * pq[h, p, d] ----
            for k in range(8):
                ps = psp.tile([128, 512], F32, tag="tD", name="weff_ps")
                for h in range(NH):
                    nc.tensor.matmul(ps[:, h * PLEN:(h + 1) * PLEN],
                                     lhsT=wpc[:, h, k, :],
                                     rhs=pq0[:, h, :], start=True, stop=True)
                nc.scalar.copy(weff[:, k, :], ps[:, 0:NH * PLEN])

            # ---- q/kv linears + pattn + softplus, per token-quarter ----
            lin_tags = ("tA", "tD")

            def linears(nt):
                sl = slice(nt * 512, (nt + 1) * 512)
                for m in range(4):
                    ps = psp.tile([128, 512], F32, tag=lin_tags[m % 2],
                                  name="lin_ps")
                    for k in range(8):
                        nc.tensor.matmul(ps,
                                         lhsT=wqc[:, k, m * 128:(m + 1) * 128],
                                         rhs=xT[:, k, sl],
                                         start=(k == 0), stop=(k == 7))
                    nc.scalar.activation(out=lin[:, m, sl], in_=ps,
                                         func=AF.Identity, bias=bqc[:, m:m + 1],
                                         scale=1.0)
                pps = psp.tile([128, 512], F32, tag="tA", name="pat_ps")
                for k in range(8):
                    nc.tensor.matmul(pps, lhsT=weff[:, k, :], rhs=xT[:, k, sl],
                                     start=(k == 0), stop=(k == 7))
                # z = ln(1 + exp(beta*pattn + beta*be)); /beta folded into rlen
                nc.scalar.activation(out=z_cm[:, sl], in_=pps, func=AF.Exp,
                                     bias=beT[:, 0:1], scale=BETA)
                nc.scalar.activation(out=z_cm[:, sl], in_=z_cm[:, sl],
                                     func=AF.Ln, bias=1.0)

                # odd halves of q/kv shifted to partitions 0-63, per quarter
                nc.sync.dma_start(out=lin0[:, :, sl], in_=lin[D:128, :, sl])

            # ---- phase C: per-chunk transposes + state deltas ----
            def phase_C(c):
                tok = slice(c * C, (c + 1) * C)
                tp = psp.tile([128, 3, C], BF16, tag="tT", name="tp")
                nc.tensor.matmul(tp[:, 0, :], lhsT=z_cm[:, tok], rhs=id128,
                                 start=True, stop=True, is_transpose=True)
                for g in range(2):
                    nc.tensor.matmul(tp[:, 1 + g, :], lhsT=lin[:, 2 + g, tok],
                                     rhs=id128, start=True, stop=True,
                                     is_transpose=True)
                nc.scalar.copy(zk[:, c, :, :], tp)

                dsp = psp.tile([128, 192], F32, tag="tT", name="dsp")
                for h in range(NH):
                    g, half = h // 2, h % 2
                    kvs = zk[:, c, 1 + g, 64 * half:64 * half + D]
                    zs = zk[:, c, 0, h * PLEN:(h + 1) * PLEN]
                    # dS1[d, (h,p)]
                    nc.tensor.matmul(
                        dsp[0:D, 64 + h * PLEN:64 + (h + 1) * PLEN],
                        lhsT=kvs, rhs=zs, start=True, stop=True)
                    # dS2[(h,p), d] (col-packed)
                    nc.tensor.matmul(dsp[32 * h:32 * h + 32, 0:D],
                                     lhsT=zs, rhs=kvs, start=True, stop=True,
                                     tile_position=(0, 32 * h))
                nc.vector.tensor_copy(dS_sb[:, c, 0:D], dsp[:, 0:D])
                nc.vector.tensor_copy(dS_sb[0:D, c, 64:192], dsp[0:D, 64:192])

            # ---- phase D: prefix sums (serial DVE chain, small) ----
            def phase_D(c):
                cur, prv = c % 2, (c - 1) % 2
                if c == 0:
                    nc.vector.tensor_copy(Scum[:, 0, :], dS_sb[:, 0, :])
                else:
                    nc.vector.tensor_add(Scum[:, cur, :], dS_sb[:, c, :],
                                         Scum[:, prv, :])
                # S1b[c] : [d, (h,p)] bf16  (prefix THROUGH c)
                nc.vector.tensor_copy(S1b[:, c, :], Scum[0:D, cur, 64:192])
                # S2b[c] : block-diagonal [(h,p), h, d]
                for h in range(NH):
                    nc.vector.tensor_copy(
                        S2b[32 * h:32 * h + 32, c, h, :],
                        Scum[32 * h:32 * h + 32, cur, 0:D])

            # ---- phase E: per-chunk attention + output ----
            def phase_E(c):
                tok = slice(c * C, (c + 1) * C)
                # M1[key, query] per head
                m1 = psp.tile([128, NH, C], F32, tag="tA", name="m1")
                for h in range(NH):
                    nc.tensor.matmul(m1[:, h, :], lhsT=kv_at0(h, tok),
                                     rhs=q_at0(h, tok), start=True, stop=True)
                m1m = work.tile([128, NH, C], BF16, tag="m1m")
                nc.vector.tensor_mul(m1m, m1, _bcast(mask, NH))

                # out1[query, (h,p)] = intra + inter (token-major direct)
                o1 = psp.tile([128, NH, PLEN], F32, tag="tU", name="o1",
                              bufs=2)
                for h in range(NH):
                    nc.tensor.matmul(o1[:, h, :],
                                     lhsT=m1m[:, h, :],
                                     rhs=zk[:, c, 0, h * PLEN:(h + 1) * PLEN],
                                     start=True, stop=(c == 0))
                    if c > 0:
                        nc.tensor.matmul(
                            o1[:, h, :],
                            lhsT=q_at0(h, tok),
                            rhs=S1b[:, c - 1, h * PLEN:(h + 1) * PLEN],
                            start=False, stop=True)
                if stage < 4:
                    return

                # softmax over plen (no max subtraction; |x| < 20 verified)
                e_sb = work.tile([128, NH, PLEN], F32, tag="e_sb")
                nc.scalar.activation(
                    out=e_sb, in_=o1,
                    func=AF.Exp, scale=rlen[:, c:c + 1])
                ssum = work.tile([128, NH], F32, tag="ssum")
                nc.vector.reduce_sum(ssum, e_sb, axis=AX.X)
                rs = work.tile([128, NH], F32, tag="rs")
                nc.vector.reciprocal(rs, ssum)
                rs2 = work.tile([128, NH], F32, tag="rs2")
                nc.vector.tensor_scalar_mul(rs2, rs, rlen[:, c:c + 1])
                aw = work.tile([128, NH, PLEN], BF16, tag="aw")
                nc.vector.tensor_mul(aw, e_sb, _bcast(rs2, PLEN, at=2))

                if stage < 5:
                    return
                # awT[(h,p), query]
                awp = psp.tile([128, C], BF16, tag="tE", name="awp")
                nc.tensor.matmul(awp, lhsT=aw.rearrange("p h w -> p (h w)"),
                                 rhs=id128, start=True, stop=True,
                                 is_transpose=True)
                awT = work.tile([128, C], BF16, tag="awT")
                nc.scalar.copy(awT, awp)

                # M2[key, query] per head (rows 32h). Disjoint row groups run
                # CONCURRENTLY in the PE array, so consecutive heads must hit
                # different PSUM banks; alternate tags (tE/tT) so the tag
                # write-after-read dependency serializes same-bank reuse.
                m2m = work.tile([128, NH, C], BF16, tag="m2m")
                for h in range(NH):
                    p0 = 32 * h
                    m2h = psp.tile([128, C], F32,
                                   tag=("tE" if h % 2 == 0 else "tV"),
                                   name=f"m2h{h % 2}")
                    nc.tensor.matmul(m2h, lhsT=z_cm[p0:p0 + 32, tok],
                                     rhs=awT[p0:p0 + 32, :],
                                     start=True, stop=True,
                                     tile_position=(p0, 0))
                    nc.vector.tensor_mul(m2m[:, h, :], m2h, mask)

                if stage < 6:
                    return
                # out2 = intra + inter, chained into one PSUM group per head
                attn = psp.tile([128, 2, C], F32, tag="tD", name="attn")
                for h in range(NH):
                    g, half = h // 2, h % 2
                    dst = attn[64 * half:64 * half + D, g, :]
                    nc.tensor.matmul(
                        dst,
                        lhsT=zk[:, c, 1 + g, 64 * half:64 * half + D],
                        rhs=m2m[:, h, :],
                        start=True, stop=(c == 0),
                        tile_position=(0, 64 * half))
                    if c > 0:
                        nc.tensor.matmul(dst, lhsT=S2b[:, c - 1, h, :],
                                         rhs=awT,
                                         start=False, stop=True,
                                         tile_position=(0, 64 * half))
                attnT = work.tile([128, 2, C], BF16, tag="attnT")
                nc.scalar.copy(attnT, attn)

                # final projection -> bf16 out (bo added on host)
                ob = outp.tile([128, EMBED], BF16, tag="ob")
                for nh in range(2):
                    osl = slice(nh * 512, (nh + 1) * 512)
                    fp = psp.tile([128, 512], F32, tag="tF", name="fp")
                    for kt in range(2):
                        nc.tensor.matmul(fp, lhsT=attnT[:, kt, :],
                                         rhs=wo[:, kt, osl],
                                         start=(kt == 0), stop=(kt == 1))
                    nc.scalar.copy(ob[:, osl], fp)
                nc.sync.dma_start(out=out_d[tok, :], in_=ob)

            # ---- interleaved emission: C/D run ahead of E by LAG chunks so
            # independent phase-C matmuls fill phase-E's dependency gaps and
            # the PE stream stays dense (keeps the HAM clock-gate warm) ----
            LAG = 3
            for q in range(4):
                linears(q)
            if stage >= 2:
                for c in range(NCH):
                    phase_C(c)
                    phase_D(c)
                    if stage >= 3 and c >= LAG:
                        phase_E(c - LAG)
            if stage >= 3:
                for c in range(NCH - LAG, NCH):
                    phase_E(c)

    nc.compile()
    return nc


_NC = None
_NC_STAGE = None


def get_nc(stage=6):
    global _NC, _NC_STAGE
    if _NC is None or _NC_STAGE != stage:
        _NC = build_nc(stage)
        _NC_STAGE = stage
    return _NC


def make_in_maps(query, pquery, Wpq, bpq, Wq, bq, Wpc, bpc, Wc, bc, Wo, bo):
    query = np.asarray(query, np.float32)
    pquery = np.asarray(pquery, np.float32)
    Wpq, Wq, Wpc, Wc, Wo = (np.asarray(w, np.float32)
                            for w in (Wpq, Wq, Wpc, Wc, Wo))
    bpq_, bq_, bpc_, bc_ = (np.asarray(v, np.float32)
                            for v in (bpq, bq, bpc, bc))
    n_idx = np.arange(NTOK, dtype=np.float64)
    rlen = (1.0 / ((n_idx + 1.0) * BETA)).astype(np.float32)
    rlen = np.ascontiguousarray(rlen.reshape(NCH, C).T)          # [C, NCH]
    mask = np.triu(np.ones((C, C), np.float32))                  # keep j <= i
    id128 = np.eye(128, dtype=np.float32)

    bf = ml_dtypes.bfloat16
    in_maps = []
    for core in range(8):
        b, hb = core // 4, core % 4
        ch = slice(hb * NH * D, (hb + 1) * NH * D)
        wqcT = np.concatenate([SCALING * Wq[ch], Wc[ch]], axis=0).T
        bqc = np.concatenate([SCALING * bq_[ch], bc_[ch]])       # (512,)
        bpqs = SCALING * bpq_[ch]                                # (256,)
        wpcR = np.ascontiguousarray(
            Wpc[ch].reshape(NH, D, 8, 128).transpose(1, 0, 2, 3))

        smf = np.zeros((128, SF_COLS), np.float32)
        smf[:, SF_BQC:SF_BQC + 4] = bqc.reshape(4, 128).T
        smf[0:D, SF_BPQ:SF_BPQ + NH] = bpqs.reshape(NH, D).T
        smf[:, SF_RLEN:SF_RLEN + NCH] = rlen
        smf[:, SF_MASK:SF_MASK + C] = mask

        smb = np.zeros((128, SB_COLS), np.float32)
        smb[:, SB_ID128:SB_ID128 + 128] = id128
        smb[0:D, SB_BPC:SB_BPC + NH] = bpc_[ch].reshape(NH, D).T

        in_maps.append({
            "xT": np.ascontiguousarray(query[:, b, :].T).astype(bf),
            "pxT": np.ascontiguousarray(pquery[:, b, :].T).astype(bf),
            "wqcT": np.ascontiguousarray(wqcT).astype(bf),
            "wpqT": np.ascontiguousarray((SCALING * Wpq[ch]).T).astype(bf),
            "wpcR": wpcR.astype(bf),
            "woT": np.ascontiguousarray(Wo[:, ch].T).astype(bf),
            "smf": smf,
            "smb": smb.astype(bf),
        })
    return in_maps


def kernel(**inputs):
    from concourse.bass_utils import run_bass_kernel_spmd
    nc = get_nc()
    in_maps = make_in_maps(**inputs)
    res = run_bass_kernel_spmd(nc, in_maps, core_ids=list(range(8)))
    bo = np.asarray(inputs["bo"], np.float32)
    out = np.zeros((NTOK, BSZ, EMBED), np.float32)
    for b in range(BSZ):
        acc = res.results[4 * b]["out"].astype(np.float32)
        for i in range(1, 4):
            acc = acc + res.results[4 * b + i]["out"].astype(np.float32)
        out[:, b, :] = acc + bo
    return out
